# revision 14
# baseline (speedup 1.0000x reference)
"""Trainium2 Bass kernel for nn_JSONTreeLSTM (K=8192, L=128, D=64) on 8 NeuronCores.

Strategy
--------
Data-parallel over K: each core gets 1024 rows of x, split into 2
software-pipelined k-groups of width 512. The NumberEmbedder is rank-1
(emb = x*w + b), so the LSTM input projection and all gate biases fold into
two extra contraction rows of the recurrent matmul (row 64 = x_t scaled by
u = W_ih_h@w_num; row 96 = ones carrying the combined bias; SBUF APs may
only start at partitions 0/32/64/96, hence the padded [97, W] rhs).

On-device math is bf16 (1 cyc/row matmuls vs 4 for fp32; 2-4x DVE modes)
accumulating in fp32 PSUM; x streams as fp8-e4m3 (its rank-1 gate
contribution u*x_t is chained into the same PSUM group as fp8 matmuls,
with u carried in row L of the fp8 xT tensor). Per step and group:

  pg[:, 0:512]   = SA.T @ rh + uA.T @ x_t   ([i; f] pre-acts + bias)
  pg[:, 512:1024]= SB.T @ rh + uB.T @ x_t   ([o; 2g])
  sab = sigmoid(pg)                (one ACT op [128, 1024]; sigma(2g))
  tg  = 2*sab[2g] - 1              (= tanh(g); DVE tensor_scalar)
  pa, pb = si*tg, sf*C             (DVE muls; both SBUF inputs of a
                                    TensorTensor must share base partition)
  T[64:128] = pa + pb              (cell state update)
  th  = tanh(C)                    (ACT [64,512])
  rh_next = sab[o] * th            (DVE)

The object-level reduction needs only per-core partials sum_k(h_L) and
sum_k(sigmoid(f)*c) with host-composed weights (W_fh@W_aout); the tiny
[1,64] object LSTM tail runs on host in float64.

Dispatch
--------
The PJRT/axon round trip dominates wall time (~80 ms floor per call plus
~9 ms/MB of input), so: the jitted 8-core shard_map executable is built
ONCE at module import and warmed with a dummy run; x ships as fp8 (1 MB
instead of 4, cast via a 64K LUT); each kernel() call then only pays
transfer + execute + one result fetch.
"""

import os
import sys
import time
from concurrent.futures import ThreadPoolExecutor

import numpy as np

sys.path.insert(0, "/opt/trn_rl_repo")

import concourse.mybir as mybir
import concourse.tile as tile
from concourse import bacc, bass_utils
import ml_dtypes

BF = ml_dtypes.bfloat16
K, L, D = 8192, 128, 64
NCORES = 8
KSH = K // NCORES      # 1024 rows per core
NG = 2                 # k-groups per core
W = KSH // NG          # 512
F32 = mybir.dt.float32
BF16 = mybir.dt.bfloat16
FP8 = mybir.dt.float8e4
F8 = mybir.dt.np(mybir.dt.float8e4)
AF = mybir.ActivationFunctionType
ALU = mybir.AluOpType

_CACHE: dict = {}


def _sigmoid(z):
    with np.errstate(over="ignore"):
        return 1.0 / (1.0 + np.exp(-z))


def _prep_weights(inp):
    """Compose SA/SB [97,128], SF [97,64] (bf16) from the model weights."""
    f = {k: np.asarray(v, np.float64) for k, v in inp.items()}
    W_ih_h = f["W_ih"][:, :D]
    u = W_ih_h @ f["W_num"][:, 0]
    bias = f["b_ih"] + f["b_hh"] + W_ih_h @ f["b_num"]
    W_hh = f["W_hh"]
    SA = np.zeros((97, 128))
    SA[0:64] = W_hh[0:128].T
    SA[96] = bias[0:128]
    SB = np.zeros((97, 128))
    SB[0:64, 0:64] = W_hh[192:256].T
    SB[0:64, 64:128] = 2.0 * W_hh[128:192].T
    SB[96] = np.concatenate([bias[192:256], 2.0 * bias[128:192]])
    Wcomb = f["W_fh"] @ f["W_aout"]
    bias_f = f["W_fh"] @ f["b_aout"] + f["b_fh"]
    SF = np.zeros((97, 64))
    SF[0:64] = Wcomb.T
    SF[96] = bias_f
    SW = np.concatenate([SA, SB, np.concatenate(
        [SF, np.zeros((97, 64))], axis=1)], axis=1)   # [97, 384]
    # u rows travel in the fp8 xT tensor (row L): [u_A(128) | u_B(128) | 0...]
    urow = np.zeros(KSH)
    urow[0:128] = u[0:128]
    urow[128:256] = np.concatenate([u[192:256], 2.0 * u[128:192]])
    return np.ascontiguousarray(SW.astype(BF)), urow


def _build_nc(n_steps=L):
    nc = bacc.Bacc("TRN2")
    xT_d = nc.dram_tensor("xT", [L + 1, KSH], FP8, kind="ExternalInput")
    SW_d = nc.dram_tensor("SW", [97, 384], BF16, kind="ExternalInput")
    out_d = nc.dram_tensor("out", [64, 4], F32, kind="ExternalOutput")

    with tile.TileContext(nc) as tc:
        with (
            tc.tile_pool(name="singles", bufs=1) as singles,
            tc.tile_pool(name="sab", bufs=3) as sab_pool,
            tc.tile_pool(name="pp", bufs=2) as p_pool,
            tc.tile_pool(name="th", bufs=2) as th_pool,
            tc.tile_pool(name="fin", bufs=1) as fin_pool,
            tc.tile_pool(name="psum", bufs=2, space="PSUM") as psum_pool,
        ):
            sw = singles.tile([97, 384], BF16, tag="sw")
            nc.sync.dma_start(sw, SW_d[:, :])
            sa = sw[:, 0:128]
            sb = sw[:, 128:256]
            sfw = sw[:, 256:320]
            ux = singles.tile([1, 256], FP8, tag="ux")
            nc.sync.dma_start(ux, xT_d[L:L + 1, 0:256])
            uxa = ux[0:1, 0:128]
            uxb = ux[0:1, 128:256]

            # T[g]: [128, W] — rows 0:64 scratch tg=tanh(g), rows 64:128 C
            TS = []
            RH = []   # RH[g][i]: [97, W] bf16 rhs (h | x | ones)
            for g in range(NG):
                T_g = singles.tile([128, W], BF16, tag=f"T{g}", name=f"T{g}")
                nc.vector.memset(T_g[:, :], 0.0)
                TS.append(T_g)
                rhs_g = [singles.tile([97, W], BF16, tag=f"rh{g}_{i}",
                                      name=f"rh{g}_{i}")
                         for i in range(3)]
                nc.vector.memset(rhs_g[0][0:64, :], 0.0)
                for i in range(3):
                    nc.vector.memset(rhs_g[i][64:96, :], 0.0)
                    nc.vector.memset(rhs_g[i][96:97, :], 1.0)
                RH.append(rhs_g)
            XB = []   # XB[g][i]: [1, W] fp8 x_t row buffers
            for g in range(NG):
                XB.append([singles.tile([1, W], FP8, tag=f"xb{g}_{i}",
                                        name=f"xb{g}_{i}")
                           for i in range(3)])

            for t in range(n_steps):
                for g in range(NG):
                    gs = g * W
                    rh_t = RH[g][t % 3]
                    rh_n = RH[g][(t + 1) % 3]
                    T = TS[g]
                    xrow = XB[g][t % 3]
                    nc.sync.dma_start(xrow, xT_d[t:t + 1, gs:gs + W])
                    pg = psum_pool.tile([128, 2 * W], F32, tag=f"pg{g}")
                    nc.tensor.matmul(pg[:, 0:W], sa, rh_t, start=True,
                                     stop=False)
                    nc.tensor.matmul(pg[:, 0:W], uxa, xrow, start=False,
                                     stop=True)
                    nc.tensor.matmul(pg[:, W:2 * W], sb, rh_t, start=True,
                                     stop=False)
                    nc.tensor.matmul(pg[:, W:2 * W], uxb, xrow, start=False,
                                     stop=True)
                    sab = sab_pool.tile([128, 2 * W], BF16, tag=f"sab{g}")
                    nc.scalar.activation(sab[:, :], pg[:, :], AF.Sigmoid)
                    # tg = 2*sigmoid(2g)-1 = tanh(g). NOTE: TensorTensor
                    # requires both SBUF inputs at the SAME base partition,
                    # so products are formed in base-0 tiles (pa, pb).
                    tg = th_pool.tile([64, W], BF16, tag=f"tg{g}",
                                      name=f"tg{g}")
                    nc.vector.tensor_scalar(
                        tg, sab[64:128, W:2 * W], 2.0, 1.0,
                        ALU.mult, ALU.subtract)
                    pa = p_pool.tile([64, W], BF16, tag=f"pa{g}",
                                     name=f"pa{g}")
                    pb = p_pool.tile([64, W], BF16, tag=f"pb{g}",
                                     name=f"pb{g}")
                    nc.vector.tensor_mul(pa, sab[0:64, 0:W], tg)
                    nc.vector.tensor_mul(pb, sab[64:128, 0:W], T[64:128, :])
                    nc.vector.tensor_add(T[64:128, :], pa, pb)
                    th = th_pool.tile([64, W], BF16, tag=f"th{g}")
                    nc.scalar.activation(th, T[64:128, :], AF.Tanh)
                    nc.vector.tensor_mul(rh_n[0:64, :], sab[0:64, W:2 * W],
                                         th)

            # ---- final per-core partials ----
            for g in range(NG):
                rh_f = RH[g][n_steps % 3]
                pf = psum_pool.tile([64, W], F32, tag=f"pg{g}")
                nc.tensor.matmul(pf, sfw, rh_f, start=True, stop=True)
                sf = fin_pool.tile([128, W], BF16, tag=f"sf{g}")
                nc.scalar.activation(sf[64:128, :], pf, AF.Sigmoid)
                scr = fin_pool.tile([64, W], BF16, tag=f"scr{g}")
                fcs = fin_pool.tile([64, 1], F32, tag=f"fcs{g}")
                nc.vector.scalar_tensor_tensor(
                    scr, sf[64:128, :], 1.0, TS[g][64:128, :], ALU.mult,
                    ALU.mult, accum_out=fcs)
                hs = fin_pool.tile([64, 1], F32, tag=f"hs{g}")
                nc.vector.tensor_reduce(hs, rh_f[0:64, :],
                                        mybir.AxisListType.X, ALU.add)
                nc.sync.dma_start(out_d[:, g:g + 1], hs)
                nc.sync.dma_start(out_d[:, 2 + g:3 + g], fcs)

    nc.finalize()
    return nc


def _get_nc(n_steps=L):
    key = ("nc", n_steps)
    if key not in _CACHE:
        _CACHE[key] = _build_nc(n_steps)
    return _CACHE[key]


def _build_dispatch():
    """One-time: jitted 8-core shard_map executable over the bass program.

    Mirrors bass_utils.run_bass_kernel_spmd's axon path (bass2jax
    run_bass_via_pjrt) but caches the jitted callable so repeat calls skip
    retrace/recompile/NEFF-reload.
    """
    import jax
    from jax.sharding import Mesh, PartitionSpec
    from jax.experimental.shard_map import shard_map
    from concourse.bass2jax import (_bass_exec_p, install_neuronx_cc_hook,
                                    partition_id_tensor)

    nc = _get_nc()
    install_neuronx_cc_hook()
    partition_name = (nc.partition_id_tensor.name
                      if nc.partition_id_tensor else None)
    in_names, out_names, out_avals, zero_shapes = [], [], [], []
    for alloc in nc.m.functions[0].allocations:
        if not isinstance(alloc, mybir.MemoryLocationSet):
            continue
        name = alloc.memorylocations[0].name
        if alloc.kind == "ExternalInput":
            if name != partition_name:
                in_names.append(name)
        elif alloc.kind == "ExternalOutput":
            out_names.append(name)
            shape = tuple(alloc.tensor_shape)
            dtype = mybir.dt.np(alloc.dtype)
            out_avals.append(jax.core.ShapedArray(shape, dtype))
            zero_shapes.append((shape, dtype))
    n_params = len(in_names)
    n_outs = len(out_avals)
    all_in_names = list(in_names) + list(out_names)
    if partition_name is not None:
        all_in_names.append(partition_name)
    donate = tuple(range(n_params, n_params + n_outs))

    def _body(*args):
        operands = list(args)
        if partition_name is not None:
            operands.append(partition_id_tensor())
        outs = _bass_exec_p.bind(
            *operands,
            out_avals=tuple(out_avals),
            in_names=tuple(all_in_names),
            out_names=tuple(out_names),
            lowering_input_output_aliases=(),
            sim_require_finite=True,
            sim_require_nnan=True,
            nc=nc,
        )
        return tuple(outs)

    devices = jax.devices()[:NCORES]
    mesh = Mesh(np.asarray(devices), ("core",))
    in_specs = (PartitionSpec("core"),) * (n_params + n_outs)
    out_specs = (PartitionSpec("core"),) * len(out_names)
    sharded = jax.jit(
        shard_map(_body, mesh=mesh, in_specs=in_specs, out_specs=out_specs,
                  check_rep=False),
        donate_argnums=donate, keep_unused=True)
    return sharded, in_names, out_names, zero_shapes


def _get_dispatch():
    if "dispatch" not in _CACHE:
        _CACHE["dispatch"] = _build_dispatch()
    return _CACHE["dispatch"]


_DUMMY_SHAPES = {
    "xT": ((L + 1, KSH), F8),
    "SW": ((97, 384), BF),
}


def _warmup():
    """Trace+compile+NEFF-load once so the first real call is cheap."""
    if _CACHE.get("warm"):
        return
    sharded, in_names, out_names, zero_shapes = _get_dispatch()
    gin = []
    for n in in_names:
        shape, dt = _DUMMY_SHAPES[n]
        gin.append(np.zeros((NCORES * shape[0], *shape[1:]), dt))
    gzero = [np.zeros((NCORES * s[0], *s[1:]), d) for (s, d) in zero_shapes]
    out = sharded(*gin, *gzero)
    np.asarray(out[0])
    _CACHE["warm"] = True


def _f8_lut():
    if "f8lut" not in _CACHE:
        with np.errstate(invalid="ignore", over="ignore"):
            bits = np.arange(65536, dtype=np.uint16)
            _CACHE["f8lut"] = (bits.view(BF).astype(np.float32)
                               .astype(F8).view(np.uint8))
    return _CACHE["f8lut"]


def _f8_cast(a):
    """Fast fp32 -> fp8e4m3 (RNE via bf16 + 64K LUT); ~3x faster than astype."""
    v = np.ascontiguousarray(a, np.float32).view(np.uint32)
    b = ((v + 0x7FFF + ((v >> 16) & 1)) >> 16).astype(np.uint16)
    return _f8_lut()[b].view(F8)


def _xt_core(x_core, urow):
    xs = np.empty((L + 1, KSH), F8)
    xs[0:L] = _f8_cast(x_core).T
    xs[L] = urow.astype(F8)
    return xs


def _run_device(x, SW, urow, trace=False, n_steps=L):
    if trace:
        # profiling path: full run_bass_kernel_spmd with NTFF trace
        nc = _get_nc(n_steps)
        in_maps = []
        for c in range(NCORES):
            in_maps.append({"xT": _xt_core(x[c * KSH:(c + 1) * KSH], urow),
                            "SW": SW})
        t0 = time.time()
        res = bass_utils.run_bass_kernel_spmd(
            nc, in_maps, core_ids=list(range(NCORES)), trace=True)
        _run_device.last_wall_s = time.time() - t0
        out = np.stack([np.asarray(r["out"], np.float64) for r in res.results])
        return out, res.exec_time_ns

    _warmup()
    sharded, in_names, out_names, zero_shapes = _get_dispatch()
    t0 = time.time()
    xg = np.empty((NCORES, L + 1, KSH), F8)
    lut = _f8_lut()

    def _one(c):
        v = np.ascontiguousarray(x[c * KSH:(c + 1) * KSH]).view(np.uint32)
        b = ((v + 0x7FFF + ((v >> 16) & 1)) >> 16).astype(np.uint16)
        xg[c, 0:L] = lut[b].view(F8).T

    if "pool" not in _CACHE:
        _CACHE["pool"] = ThreadPoolExecutor(NCORES)
    list(_CACHE["pool"].map(_one, range(NCORES)))
    xg[:, L] = urow.astype(F8)
    xg = xg.reshape(NCORES * (L + 1), KSH)
    wmap = {"SW": SW}
    gin = []
    for name in in_names:
        if name == "xT":
            gin.append(xg)
        else:
            gin.append(np.concatenate([wmap[name]] * NCORES, axis=0))
    gzero = [np.zeros((NCORES * s[0], *s[1:]), d) for (s, d) in zero_shapes]
    out_arrs = sharded(*gin, *gzero)
    out0 = np.asarray(out_arrs[0])
    _run_device.last_wall_s = time.time() - t0
    return out0.reshape(NCORES, 64, 4).astype(np.float64), None


def kernel(**inputs):
    inp = {k: np.asarray(v) for k, v in inputs.items()}
    SW, urow = _prep_weights(inp)
    x = np.ascontiguousarray(np.asarray(inp["x"], np.float32))
    trace = bool(int(os.environ.get("LSTM_TRACE", "0")))
    out, exec_ns = _run_device(x, SW, urow, trace=trace)
    kernel._last_exec_ns = exec_ns
    hsum = out[:, :, 0].sum(0) + out[:, :, 1].sum(0)
    fcs = out[:, :, 2].sum(0) + out[:, :, 3].sum(0)
    # ---- host: object-level TreeLSTM tail (tiny) ----
    f = {k: np.asarray(v, np.float64) for k, v in inp.items()}
    hs_bar = hsum @ f["W_aout"].T + K * f["b_aout"]
    iou = hs_bar @ f["W_iouh"].T + f["b_iouh"]
    i, o_, u = iou[0:64], iou[64:128], iou[128:192]
    c_obj = _sigmoid(i) * np.tanh(u) + fcs
    h_obj = _sigmoid(o_) * np.tanh(c_obj)
    h_hat = h_obj @ f["W_oout"].T + f["b_oout"]
    return np.concatenate([h_hat, c_obj])[None].astype(np.float32)


kernel._last_exec_ns = None
_run_device.last_wall_s = None

# Import-time warmup: build + compile + load the executable so the first
# kernel() call only pays transfer + execute. Never let warmup failure
# break the import; kernel() will retry lazily.
if not bool(int(os.environ.get("LSTM_NO_WARMUP", "0"))):
    try:
        _warmup()
    except Exception:
        _CACHE.pop("warm", None)


# revision 16
# speedup vs baseline: 1.2832x; 1.2832x over previous
"""Trainium2 Bass kernel for nn_JSONTreeLSTM (K=8192, L=128, D=64) on 8 NeuronCores.

Strategy
--------
Data-parallel over K: each core gets 1024 rows of x, split into 2
software-pipelined k-groups of width 512. The NumberEmbedder is rank-1
(emb = x*w + b), so the LSTM input projection and all gate biases fold into
two extra contraction rows of the recurrent matmul (row 64 = x_t scaled by
u = W_ih_h@w_num; row 96 = ones carrying the combined bias; SBUF APs may
only start at partitions 0/32/64/96, hence the padded [97, W] rhs).

On-device math is bf16 (1 cyc/row matmuls vs 4 for fp32; 2-4x DVE modes)
accumulating in fp32 PSUM; x streams as fp8-e4m3 (its rank-1 gate
contribution u*x_t is chained into the same PSUM group as fp8 matmuls,
with u carried in row L of the fp8 xT tensor). Per step and group:

  pg[:, 0:512]   = SA.T @ rh + uA.T @ x_t   ([i; f] pre-acts + bias)
  pg[:, 512:1024]= SB.T @ rh + uB.T @ x_t   ([o; 2g])
  sab = sigmoid(pg)                (one ACT op [128, 1024]; sigma(2g))
  tg  = 2*sab[2g] - 1              (= tanh(g); DVE tensor_scalar)
  pa, pb = si*tg, sf*C             (DVE muls; both SBUF inputs of a
                                    TensorTensor must share base partition)
  T[64:128] = pa + pb              (cell state update)
  th  = tanh(C)                    (ACT [64,512])
  rh_next = sab[o] * th            (DVE)

The object-level reduction needs only per-core partials sum_k(h_L) and
sum_k(sigmoid(f)*c) with host-composed weights (W_fh@W_aout); the tiny
[1,64] object LSTM tail runs on host in float64.

Dispatch
--------
The PJRT/axon round trip dominates wall time (~80 ms floor per call plus
~9 ms/MB of input), so: the jitted 8-core shard_map executable is built
ONCE at module import and warmed with a dummy run; x ships as fp8 (1 MB
instead of 4, cast via a 64K LUT); each kernel() call then only pays
transfer + execute + one result fetch.
"""

import os
import sys
import time
from concurrent.futures import ThreadPoolExecutor

import numpy as np

sys.path.insert(0, "/opt/trn_rl_repo")

import concourse.mybir as mybir
import concourse.tile as tile
from concourse import bacc, bass_utils
import ml_dtypes

BF = ml_dtypes.bfloat16
K, L, D = 8192, 128, 64
NCORES = 8
KSH = K // NCORES      # 1024 rows per core
NG = 2                 # k-groups per core
W = KSH // NG          # 512
F32 = mybir.dt.float32
BF16 = mybir.dt.bfloat16
FP8 = mybir.dt.float8e4
F8 = mybir.dt.np(mybir.dt.float8e4)
AF = mybir.ActivationFunctionType
ALU = mybir.AluOpType

_CACHE: dict = {}


def _sigmoid(z):
    with np.errstate(over="ignore"):
        return 1.0 / (1.0 + np.exp(-z))


def _prep_weights(inp):
    """Compose SA/SB [97,128], SF [97,64] (bf16) from the model weights."""
    f = {k: np.asarray(v, np.float64) for k, v in inp.items()}
    W_ih_h = f["W_ih"][:, :D]
    u = W_ih_h @ f["W_num"][:, 0]
    bias = f["b_ih"] + f["b_hh"] + W_ih_h @ f["b_num"]
    W_hh = f["W_hh"]
    SA = np.zeros((97, 128))
    SA[0:64] = W_hh[0:128].T
    SA[96] = bias[0:128]
    SB = np.zeros((97, 128))
    SB[0:64, 0:64] = W_hh[192:256].T
    SB[0:64, 64:128] = 2.0 * W_hh[128:192].T
    SB[96] = np.concatenate([bias[192:256], 2.0 * bias[128:192]])
    Wcomb = f["W_fh"] @ f["W_aout"]
    bias_f = f["W_fh"] @ f["b_aout"] + f["b_fh"]
    SF = np.zeros((97, 64))
    SF[0:64] = Wcomb.T
    SF[96] = bias_f
    SW = np.concatenate([SA, SB, np.concatenate(
        [SF, np.zeros((97, 64))], axis=1)], axis=1)   # [97, 384]
    # u rows travel in the fp8 xT tensor (row L): [u_A(128) | u_B(128) | 0...]
    urow = np.zeros(KSH)
    urow[0:128] = u[0:128]
    urow[128:256] = np.concatenate([u[192:256], 2.0 * u[128:192]])
    return np.ascontiguousarray(SW.astype(BF)), urow


def _build_nc(n_steps=L):
    nc = bacc.Bacc("TRN2")
    xT_d = nc.dram_tensor("xT", [L + 1, KSH], FP8, kind="ExternalInput")
    SW_d = nc.dram_tensor("SW", [97, 384], BF16, kind="ExternalInput")
    out_d = nc.dram_tensor("out", [64, 4], F32, kind="ExternalOutput")

    with tile.TileContext(nc) as tc:
        with (
            tc.tile_pool(name="singles", bufs=1) as singles,
            tc.tile_pool(name="sab", bufs=3) as sab_pool,
            tc.tile_pool(name="pp", bufs=2) as p_pool,
            tc.tile_pool(name="th", bufs=2) as th_pool,
            tc.tile_pool(name="fin", bufs=1) as fin_pool,
            tc.tile_pool(name="psum", bufs=2, space="PSUM") as psum_pool,
        ):
            sw = singles.tile([97, 384], BF16, tag="sw")
            nc.sync.dma_start(sw, SW_d[:, :])
            sa = sw[:, 0:128]
            sb = sw[:, 128:256]
            sfw = sw[:, 256:320]
            ux = singles.tile([1, 256], FP8, tag="ux")
            nc.sync.dma_start(ux, xT_d[L:L + 1, 0:256])
            uxa = ux[0:1, 0:128]
            uxb = ux[0:1, 128:256]

            # T[g]: [128, W] — rows 0:64 scratch tg=tanh(g), rows 64:128 C
            TS = []
            RH = []   # RH[g][i]: [97, W] bf16 rhs (h | x | ones)
            for g in range(NG):
                T_g = singles.tile([128, W], BF16, tag=f"T{g}", name=f"T{g}")
                nc.vector.memset(T_g[:, :], 0.0)
                TS.append(T_g)
                rhs_g = [singles.tile([97, W], BF16, tag=f"rh{g}_{i}",
                                      name=f"rh{g}_{i}")
                         for i in range(3)]
                nc.vector.memset(rhs_g[0][0:64, :], 0.0)
                for i in range(3):
                    nc.vector.memset(rhs_g[i][64:96, :], 0.0)
                    nc.vector.memset(rhs_g[i][96:97, :], 1.0)
                RH.append(rhs_g)
            XB = []   # XB[g][i]: [1, W] fp8 x_t row buffers
            for g in range(NG):
                XB.append([singles.tile([1, W], FP8, tag=f"xb{g}_{i}",
                                        name=f"xb{g}_{i}")
                           for i in range(3)])

            for t in range(n_steps):
                for g in range(NG):
                    gs = g * W
                    rh_t = RH[g][t % 3]
                    rh_n = RH[g][(t + 1) % 3]
                    T = TS[g]
                    xrow = XB[g][t % 3]
                    nc.sync.dma_start(xrow, xT_d[t:t + 1, gs:gs + W])
                    pg = psum_pool.tile([128, 2 * W], F32, tag=f"pg{g}")
                    nc.tensor.matmul(pg[:, 0:W], sa, rh_t, start=True,
                                     stop=False)
                    nc.tensor.matmul(pg[:, 0:W], uxa, xrow, start=False,
                                     stop=True)
                    nc.tensor.matmul(pg[:, W:2 * W], sb, rh_t, start=True,
                                     stop=False)
                    nc.tensor.matmul(pg[:, W:2 * W], uxb, xrow, start=False,
                                     stop=True)
                    sab = sab_pool.tile([128, 2 * W], BF16, tag=f"sab{g}")
                    nc.scalar.activation(sab[:, :], pg[:, :], AF.Sigmoid)
                    # tg = 2*sigmoid(2g)-1 = tanh(g). NOTE: TensorTensor
                    # requires both SBUF inputs at the SAME base partition,
                    # so products are formed in base-0 tiles (pa, pb).
                    tg = th_pool.tile([64, W], BF16, tag=f"tg{g}",
                                      name=f"tg{g}")
                    nc.vector.tensor_scalar(
                        tg, sab[64:128, W:2 * W], 2.0, 1.0,
                        ALU.mult, ALU.subtract)
                    pa = p_pool.tile([64, W], BF16, tag=f"pa{g}",
                                     name=f"pa{g}")
                    pb = p_pool.tile([64, W], BF16, tag=f"pb{g}",
                                     name=f"pb{g}")
                    nc.vector.tensor_mul(pa, sab[0:64, 0:W], tg)
                    nc.vector.tensor_mul(pb, sab[64:128, 0:W], T[64:128, :])
                    nc.vector.tensor_add(T[64:128, :], pa, pb)
                    th = th_pool.tile([64, W], BF16, tag=f"th{g}")
                    nc.scalar.activation(th, T[64:128, :], AF.Tanh)
                    nc.vector.tensor_mul(rh_n[0:64, :], sab[0:64, W:2 * W],
                                         th)

            # ---- final per-core partials ----
            for g in range(NG):
                rh_f = RH[g][n_steps % 3]
                pf = psum_pool.tile([64, W], F32, tag=f"pg{g}")
                nc.tensor.matmul(pf, sfw, rh_f, start=True, stop=True)
                sf = fin_pool.tile([128, W], BF16, tag=f"sf{g}")
                nc.scalar.activation(sf[64:128, :], pf, AF.Sigmoid)
                scr = fin_pool.tile([64, W], BF16, tag=f"scr{g}")
                fcs = fin_pool.tile([64, 1], F32, tag=f"fcs{g}")
                nc.vector.scalar_tensor_tensor(
                    scr, sf[64:128, :], 1.0, TS[g][64:128, :], ALU.mult,
                    ALU.mult, accum_out=fcs)
                hs = fin_pool.tile([64, 1], F32, tag=f"hs{g}")
                nc.vector.tensor_reduce(hs, rh_f[0:64, :],
                                        mybir.AxisListType.X, ALU.add)
                nc.sync.dma_start(out_d[:, g:g + 1], hs)
                nc.sync.dma_start(out_d[:, 2 + g:3 + g], fcs)

    nc.finalize()
    return nc


def _get_nc(n_steps=L):
    key = ("nc", n_steps)
    if key not in _CACHE:
        _CACHE[key] = _build_nc(n_steps)
    return _CACHE[key]


def _build_dispatch():
    """One-time: jitted 8-core shard_map executable over the bass program.

    Mirrors bass_utils.run_bass_kernel_spmd's axon path (bass2jax
    run_bass_via_pjrt) but caches the jitted callable so repeat calls skip
    retrace/recompile/NEFF-reload.
    """
    import jax
    from jax.sharding import Mesh, PartitionSpec
    from jax.experimental.shard_map import shard_map
    from concourse.bass2jax import (_bass_exec_p, install_neuronx_cc_hook,
                                    partition_id_tensor)

    nc = _get_nc()
    install_neuronx_cc_hook()
    partition_name = (nc.partition_id_tensor.name
                      if nc.partition_id_tensor else None)
    in_names, out_names, out_avals, zero_shapes = [], [], [], []
    for alloc in nc.m.functions[0].allocations:
        if not isinstance(alloc, mybir.MemoryLocationSet):
            continue
        name = alloc.memorylocations[0].name
        if alloc.kind == "ExternalInput":
            if name != partition_name:
                in_names.append(name)
        elif alloc.kind == "ExternalOutput":
            out_names.append(name)
            shape = tuple(alloc.tensor_shape)
            dtype = mybir.dt.np(alloc.dtype)
            out_avals.append(jax.core.ShapedArray(shape, dtype))
            zero_shapes.append((shape, dtype))
    n_params = len(in_names)
    n_outs = len(out_avals)
    all_in_names = list(in_names) + list(out_names)
    if partition_name is not None:
        all_in_names.append(partition_name)
    donate = tuple(range(n_params, n_params + n_outs))

    def _body(*args):
        operands = list(args)
        if partition_name is not None:
            operands.append(partition_id_tensor())
        outs = _bass_exec_p.bind(
            *operands,
            out_avals=tuple(out_avals),
            in_names=tuple(all_in_names),
            out_names=tuple(out_names),
            lowering_input_output_aliases=(),
            sim_require_finite=True,
            sim_require_nnan=True,
            nc=nc,
        )
        return tuple(outs)

    devices = jax.devices()[:NCORES]
    mesh = Mesh(np.asarray(devices), ("core",))
    in_specs = (PartitionSpec("core"),) * (n_params + n_outs)
    out_specs = (PartitionSpec("core"),) * len(out_names)
    sharded = jax.jit(
        shard_map(_body, mesh=mesh, in_specs=in_specs, out_specs=out_specs,
                  check_rep=False),
        donate_argnums=donate, keep_unused=True)
    from jax.sharding import NamedSharding
    rowsh = NamedSharding(mesh, PartitionSpec("core"))
    return sharded, in_names, out_names, zero_shapes, rowsh


def _get_dispatch():
    if "dispatch" not in _CACHE:
        _CACHE["dispatch"] = _build_dispatch()
    return _CACHE["dispatch"]


_DUMMY_SHAPES = {
    "xT": ((L + 1, KSH), F8),
    "SW": ((97, 384), BF),
}


def _warmup():
    """Trace+compile+NEFF-load once so the first real call is cheap.

    Exercises the exact dispatch path of _run_device (async device_put with
    the row sharding + jitted call on device-committed arrays) so the first
    real call pays no one-time lowering.
    """
    if _CACHE.get("warm"):
        return
    import jax
    sharded, in_names, out_names, zero_shapes, rowsh = _get_dispatch()
    gin = []
    for n in in_names:
        shape, dt = _DUMMY_SHAPES[n]
        gin.append(jax.device_put(
            np.zeros((NCORES * shape[0], *shape[1:]), dt), rowsh))
    gzero = [jax.device_put(np.zeros((NCORES * s[0], *s[1:]), d), rowsh)
             for (s, d) in zero_shapes]
    out = sharded(*gin, *gzero)
    np.asarray(out[0])
    _CACHE["warm"] = True


def _f8_lut():
    if "f8lut" not in _CACHE:
        with np.errstate(invalid="ignore", over="ignore"):
            bits = np.arange(65536, dtype=np.uint16)
            _CACHE["f8lut"] = (bits.view(BF).astype(np.float32)
                               .astype(F8).view(np.uint8))
    return _CACHE["f8lut"]


def _f8_cast(a):
    """Fast fp32 -> fp8e4m3 (RNE via bf16 + 64K LUT); ~3x faster than astype."""
    v = np.ascontiguousarray(a, np.float32).view(np.uint32)
    b = ((v + 0x7FFF + ((v >> 16) & 1)) >> 16).astype(np.uint16)
    return _f8_lut()[b].view(F8)


def _xt_core(x_core, urow):
    xs = np.empty((L + 1, KSH), F8)
    xs[0:L] = _f8_cast(x_core).T
    xs[L] = urow.astype(F8)
    return xs


def _run_device(x, SW, urow, trace=False, n_steps=L):
    if trace:
        # profiling path: full run_bass_kernel_spmd with NTFF trace
        nc = _get_nc(n_steps)
        in_maps = []
        for c in range(NCORES):
            in_maps.append({"xT": _xt_core(x[c * KSH:(c + 1) * KSH], urow),
                            "SW": SW})
        t0 = time.time()
        res = bass_utils.run_bass_kernel_spmd(
            nc, in_maps, core_ids=list(range(NCORES)), trace=True)
        _run_device.last_wall_s = time.time() - t0
        out = np.stack([np.asarray(r["out"], np.float64) for r in res.results])
        return out, res.exec_time_ns

    _warmup()
    import jax
    sharded, in_names, out_names, zero_shapes, rowsh = _get_dispatch()
    t0 = time.time()
    # dispatch the small transfers first (async) so they overlap the x cast
    gzero = [jax.device_put(np.zeros((NCORES * s[0], *s[1:]), d), rowsh)
             for (s, d) in zero_shapes]
    swd = jax.device_put(np.concatenate([SW] * NCORES, axis=0), rowsh)
    xg = np.empty((NCORES, L + 1, KSH), F8)
    lut = _f8_lut()

    def _one(c):
        v = np.ascontiguousarray(x[c * KSH:(c + 1) * KSH]).view(np.uint32)
        b = ((v + 0x7FFF + ((v >> 16) & 1)) >> 16).astype(np.uint16)
        xg[c, 0:L] = lut[b].view(F8).T

    if "pool" not in _CACHE:
        _CACHE["pool"] = ThreadPoolExecutor(NCORES)
    list(_CACHE["pool"].map(_one, range(NCORES)))
    xg[:, L] = urow.astype(F8)
    xd = jax.device_put(xg.reshape(NCORES * (L + 1), KSH), rowsh)
    gin = [xd if name == "xT" else swd for name in in_names]
    out_arrs = sharded(*gin, *gzero)
    out0 = np.asarray(out_arrs[0])
    _run_device.last_wall_s = time.time() - t0
    return out0.reshape(NCORES, 64, 4).astype(np.float64), None


def kernel(**inputs):
    inp = {k: np.asarray(v) for k, v in inputs.items()}
    SW, urow = _prep_weights(inp)
    x = np.ascontiguousarray(np.asarray(inp["x"], np.float32))
    trace = bool(int(os.environ.get("LSTM_TRACE", "0")))
    out, exec_ns = _run_device(x, SW, urow, trace=trace)
    kernel._last_exec_ns = exec_ns
    hsum = out[:, :, 0].sum(0) + out[:, :, 1].sum(0)
    fcs = out[:, :, 2].sum(0) + out[:, :, 3].sum(0)
    # ---- host: object-level TreeLSTM tail (tiny) ----
    f = {k: np.asarray(v, np.float64) for k, v in inp.items()}
    hs_bar = hsum @ f["W_aout"].T + K * f["b_aout"]
    iou = hs_bar @ f["W_iouh"].T + f["b_iouh"]
    i, o_, u = iou[0:64], iou[64:128], iou[128:192]
    c_obj = _sigmoid(i) * np.tanh(u) + fcs
    h_obj = _sigmoid(o_) * np.tanh(c_obj)
    h_hat = h_obj @ f["W_oout"].T + f["b_oout"]
    return np.concatenate([h_hat, c_obj])[None].astype(np.float32)


kernel._last_exec_ns = None
_run_device.last_wall_s = None

# Import-time warmup: build + compile + load the executable so the first
# kernel() call only pays transfer + execute. Never let warmup failure
# break the import; kernel() will retry lazily.
if not bool(int(os.environ.get("LSTM_NO_WARMUP", "0"))):
    try:
        _warmup()
    except Exception:
        _CACHE.pop("warm", None)
